# revision 8
# baseline (speedup 1.0000x reference)
"""Multi-head attention Trainium2 kernel (B=8,S=1024,D=1024,H=16,DK=64).

Data-parallel over batch: one batch element per NeuronCore (8 cores).
Per core:
  1. QP = q @ w_q, KP = k @ w_k, VP = v @ w_v           (fp32 matmuls)
  2. torch-.view head split: head h of Q is QP rows [h*64,(h+1)*64)
     reinterpreted as [1024, 64] (pure reshape; row index == seq index)
  3. S^T[k,q] = K_h Q_h^T / 8 computed transposed (k on partitions) so
     softmax normalizer comes out of the PV matmul via a ones-column on V
  4. exp (no max subtraction needed: |scores| <= ~25), causal via
     skipping k>q column ranges + triangular 0/1 mask on diagonal blocks
  5. PV with V augmented by ones column -> unnormalized out + sumexp,
     normalize with reciprocal, reassemble OP (inverse .view), OP @ w_o
"""

import numpy as np

import concourse.bass as bass
import concourse.mybir as mybir
import concourse.tile as tile
from concourse import bacc
from concourse.bass_utils import run_bass_kernel_spmd
from concourse.masks import make_identity

B, S, D, H, DK = 8, 1024, 1024, 16, 64
P = 128
F32 = mybir.dt.float32
F32R = mybir.dt.float32r


def _build_nc(mm_mode: str = "f32"):
    """Build the Bass program. mm_mode: 'f32' (exact) or 'f32r' (fast fp32)."""

    # In f32r mode, tiles feeding the big matmuls (projections, scores,
    # final) are declared float32r so their producers (ACT/DVE copies, exp)
    # round on write, which the BIR verifier requires. PV + transposes stay
    # full fp32.
    MDT = F32R if mm_mode == "f32r" else F32

    def mmc(ap):
        return ap

    nc = bacc.Bacc(
        "TRN2",
        target_bir_lowering=False,
        debug=False,
        enable_asserts=False,
        num_devices=B,
    )

    q_d = nc.dram_tensor("q", [S, D], F32, kind="ExternalInput")
    k_d = nc.dram_tensor("k", [S, D], F32, kind="ExternalInput")
    v_d = nc.dram_tensor("v", [S, D], F32, kind="ExternalInput")
    wq_d = nc.dram_tensor("w_q", [D, D], F32, kind="ExternalInput")
    wk_d = nc.dram_tensor("w_k", [D, D], F32, kind="ExternalInput")
    wv_d = nc.dram_tensor("w_v", [D, D], F32, kind="ExternalInput")
    wo_d = nc.dram_tensor("w_o", [D, D], F32, kind="ExternalInput")
    out_d = nc.dram_tensor("out", [S, D], F32, kind="ExternalOutput")

    qp_d = nc.dram_tensor("qp_scratch", [S, D], F32, kind="Internal")
    kp_d = nc.dram_tensor("kp_scratch", [S, D], F32, kind="Internal")
    vp_d = nc.dram_tensor("vp_scratch", [S, D], F32, kind="Internal")

    with tile.TileContext(nc) as tc:
        with (
            tc.tile_pool(name="consts", bufs=1) as consts,
            tc.tile_pool(name="wpool", bufs=1) as wpool,
            tc.tile_pool(name="xtp", bufs=1) as xtp,
            tc.tile_pool(name="iop", bufs=3) as iop,
            tc.tile_pool(name="hp", bufs=2) as hp,
            tc.tile_pool(name="ptp", bufs=1) as ptp,
            tc.tile_pool(name="opp", bufs=1) as opp,
            tc.tile_pool(name="sp", bufs=4) as sp,
            tc.tile_pool(name="psA", bufs=3, space="PSUM") as psA,
            tc.tile_pool(name="psB", bufs=2, space="PSUM") as psB,
            tc.tile_pool(name="psC", bufs=2, space="PSUM") as psC,
        ):
            ident = consts.tile([P, P], F32, tag="ident")
            make_identity(nc, ident[:])
            # tri[k, q] = 1.0 if q >= k else 0.0  (keep causal-valid entries)
            tri = consts.tile([P, P], F32, tag="tri")
            nc.gpsimd.memset(tri[:], 1.0)
            nc.gpsimd.affine_select(
                out=tri[:],
                in_=tri[:],
                compare_op=mybir.AluOpType.is_ge,
                fill=0.0,
                base=0,
                pattern=[[1, P]],
                channel_multiplier=-1,
            )

            # ---------------- Phase 1: projections -> DRAM scratch ----------
            for x_d, w_d, xp_d in (
                (q_d, wq_d, qp_d),
                (k_d, wk_d, kp_d),
                (v_d, wv_d, vp_d),
            ):
                w_sb = wpool.tile([P, 8, 1024], MDT, tag="w")
                if MDT == F32:
                    nc.sync.dma_start(
                        w_sb[:], w_d.ap().rearrange("(kd p) c -> p kd c", p=P)
                    )
                else:
                    for kd in range(8):
                        wstg = iop.tile([P, 1024], F32, tag="wstg")
                        nc.sync.dma_start(wstg[:], w_d.ap()[kd * P : (kd + 1) * P, :])
                        nc.scalar.copy(out=w_sb[:, kd, :], in_=wstg[:])
                xt_sb = xtp.tile([P, 8, 1024], MDT, tag="xt")
                for st in range(8):
                    nat = iop.tile([P, 1024], F32, tag="nat")
                    nc.sync.dma_start(nat[:], x_d.ap()[st * P : (st + 1) * P, :])
                    for kd in range(8):
                        tp = psB.tile([P, P], F32, tag="tp")
                        nc.tensor.transpose(
                            tp[:], nat[:, kd * P : (kd + 1) * P], ident[:]
                        )
                        nc.any.tensor_copy(
                            out=xt_sb[:, kd, st * P : (st + 1) * P], in_=tp[:]
                        )
                for st in range(8):
                    for ch in range(2):
                        ps = psA.tile([P, 512], F32, tag="mm")
                        for kd in range(8):
                            nc.tensor.matmul(
                                ps[:],
                                mmc(xt_sb[:, kd, st * P : (st + 1) * P]),
                                mmc(w_sb[:, kd, ch * 512 : (ch + 1) * 512]),
                                start=(kd == 0),
                                stop=(kd == 7),
                            )
                        stg = iop.tile([P, 512], F32, tag="stg")
                        nc.any.tensor_copy(out=stg[:], in_=ps[:])
                        nc.sync.dma_start(
                            xp_d.ap()[
                                st * P : (st + 1) * P, ch * 512 : (ch + 1) * 512
                            ],
                            stg[:],
                        )

            # ------------- Phase 2: attention, one head at a time -----------
            # X_h[s, dk] = XP[h*64 + s//16, (s%16)*64 + dk] -> contiguous view
            qp_r = qp_d.ap().rearrange("(h a) (b u) -> h (a b) u", h=H, b=16)
            kp_r = kp_d.ap().rearrange("(h a) (b u) -> h (a b) u", h=H, b=16)
            vp_r = vp_d.ap().rearrange("(h a) (b u) -> h (a b) u", h=H, b=16)

            op_sb = opp.tile([P, 8, 1024], F32, tag="op")

            for h in range(H):
                qhT = hp.tile([P, 1024], MDT, tag="qhT")
                khT = hp.tile([P, 1024], MDT, tag="khT")
                nc.vector.memset(qhT[DK:P, :].bitcast(F32), 0.0)
                nc.vector.memset(khT[DK:P, :].bitcast(F32), 0.0)
                vo = hp.tile([P, 8, DK + 1], F32, tag="vo")
                nc.vector.memset(vo[:, :, DK : DK + 1], 1.0)
                for t in range(8):
                    qh = iop.tile([P, DK], F32, tag="qh")
                    nc.sync.dma_start(qh[:], qp_r[h, t * P : (t + 1) * P, :])
                    tpq = psB.tile([P, P], F32, tag="tp")
                    nc.tensor.transpose(tpq[:DK, :], qh[:], ident[:])
                    nc.any.tensor_copy(
                        out=qhT[:DK, t * P : (t + 1) * P], in_=tpq[:DK, :]
                    )
                    kh = iop.tile([P, DK], F32, tag="kh")
                    nc.sync.dma_start(kh[:], kp_r[h, t * P : (t + 1) * P, :])
                    tpk = psB.tile([P, P], F32, tag="tp")
                    nc.tensor.transpose(tpk[:DK, :], kh[:], ident[:])
                    nc.any.tensor_copy(
                        out=khT[:DK, t * P : (t + 1) * P], in_=tpk[:DK, :]
                    )
                    nc.sync.dma_start(
                        vo[:, t, :DK], vp_r[h, t * P : (t + 1) * P, :]
                    )

                # scores^T (k on partitions) + exp; only q >= k*128 ranges
                pt = ptp.tile([P, 8, 1024], F32, tag="pt")
                for j in range(8):
                    q0 = j * P
                    off = q0
                    while off < 1024:
                        n = min(512, 1024 - off)
                        ps = psA.tile([P, 512], F32, tag="mm")
                        nc.tensor.matmul(
                            ps[:, :n],
                            mmc(khT[:, q0 : q0 + P]),
                            mmc(qhT[:, off : off + n]),
                            start=True,
                            stop=True,
                        )
                        nc.scalar.activation(
                            out=pt[:, j, off : off + n],
                            in_=ps[:, :n],
                            func=mybir.ActivationFunctionType.Exp,
                            scale=0.125,
                        )
                        off += n
                    # causal mask inside the diagonal 128x128 block
                    nc.vector.tensor_tensor(
                        pt[:, j, q0 : q0 + P],
                        pt[:, j, q0 : q0 + P],
                        tri[:],
                        mybir.AluOpType.mult,
                    )

                # PV with ones-column -> [q, 64 out | sumexp], then normalize
                for i in range(8):
                    pv = psC.tile([P, DK + 1], F32, tag="pv")
                    for j in range(i + 1):
                        nc.tensor.matmul(
                            pv[:],
                            mmc(pt[:, j, i * P : (i + 1) * P]),
                            mmc(vo[:, j, :]),
                            start=(j == 0),
                            stop=(j == i),
                        )
                    rec = sp.tile([P, 1], F32, tag="rec")
                    nc.vector.reciprocal(rec[:], pv[:, DK : DK + 1])
                    stg = sp.tile([P, DK], F32, tag="ostg")
                    nc.vector.tensor_scalar_mul(stg[:], pv[:, :DK], rec[:])
                    # place into OP[h*64 + i*8 + a, b*64 + dk]  (inverse .view)
                    r0 = (h % 2) * 64 + i * 8
                    nc.sync.dma_start(
                        op_sb[r0 : r0 + 8, h // 2, :], stg[:]
                    )

            # ---------------- Phase 3: output projection --------------------
            wo_sb = wpool.tile([P, 8, 1024], MDT, tag="w")
            if MDT == F32:
                nc.sync.dma_start(
                    wo_sb[:], wo_d.ap().rearrange("(kd p) c -> p kd c", p=P)
                )
            else:
                for kd in range(8):
                    wstg = iop.tile([P, 1024], F32, tag="wstg")
                    nc.sync.dma_start(wstg[:], wo_d.ap()[kd * P : (kd + 1) * P, :])
                    nc.scalar.copy(out=wo_sb[:, kd, :], in_=wstg[:])
            opT = xtp.tile([P, 8, 1024], MDT, tag="xt")
            for m in range(8):
                for cc in range(8):
                    tp = psB.tile([P, P], F32, tag="tp")
                    nc.tensor.transpose(
                        tp[:], op_sb[:, m, cc * P : (cc + 1) * P], ident[:]
                    )
                    nc.any.tensor_copy(
                        out=opT[:, cc, m * P : (m + 1) * P], in_=tp[:]
                    )
            for mt in range(8):
                for ch in range(2):
                    ps = psA.tile([P, 512], F32, tag="mm")
                    for cd in range(8):
                        nc.tensor.matmul(
                            ps[:],
                            mmc(opT[:, cd, mt * P : (mt + 1) * P]),
                            mmc(wo_sb[:, cd, ch * 512 : (ch + 1) * 512]),
                            start=(cd == 0),
                            stop=(cd == 7),
                        )
                    stg = iop.tile([P, 512], F32, tag="stg")
                    nc.any.tensor_copy(out=stg[:], in_=ps[:])
                    nc.sync.dma_start(
                        out_d.ap()[
                            mt * P : (mt + 1) * P, ch * 512 : (ch + 1) * 512
                        ],
                        stg[:],
                    )

    if not nc.is_finalized():
        nc.finalize()
    return nc


_nc_cache = {}


def _get_nc(mm_mode):
    if mm_mode not in _nc_cache:
        _nc_cache[mm_mode] = _build_nc(mm_mode)
    return _nc_cache[mm_mode]


MM_MODE = "f32"


def kernel(q, k, v, mask, w_q, w_k, w_v, w_o, _trace=False):
    q = np.ascontiguousarray(np.asarray(q, dtype=np.float32))
    k = np.ascontiguousarray(np.asarray(k, dtype=np.float32))
    v = np.ascontiguousarray(np.asarray(v, dtype=np.float32))
    w_q = np.ascontiguousarray(np.asarray(w_q, dtype=np.float32))
    w_k = np.ascontiguousarray(np.asarray(w_k, dtype=np.float32))
    w_v = np.ascontiguousarray(np.asarray(w_v, dtype=np.float32))
    w_o = np.ascontiguousarray(np.asarray(w_o, dtype=np.float32))

    nc = _get_nc(MM_MODE)
    in_maps = [
        {
            "q": q[i],
            "k": k[i],
            "v": v[i],
            "w_q": w_q,
            "w_k": w_k,
            "w_v": w_v,
            "w_o": w_o,
        }
        for i in range(B)
    ]
    res = run_bass_kernel_spmd(
        nc, in_maps, core_ids=list(range(B)), trace=_trace
    )
    out = np.stack([r["out"] for r in res.results], axis=0)
    if _trace:
        kernel.last_exec_time_ns = res.exec_time_ns
        kernel.last_trace = res.instructions_and_trace
    return out


# revision 18
# speedup vs baseline: 5344.5683x; 5344.5683x over previous
"""Multi-head attention Trainium2 kernel (B=8,S=1024,D=1024,H=16,DK=64).

Data-parallel over batch: one batch element per NeuronCore (8 cores).
Per core:
  1. QP = q @ w_q, KP = k @ w_k, VP = v @ w_v           (fp32 matmuls)
  2. torch-.view head split: head h of Q is QP rows [h*64,(h+1)*64)
     reinterpreted as [1024, 64] (pure reshape; row index == seq index)
  3. S^T[k,q] = K_h Q_h^T / 8 computed transposed (k on partitions) so
     softmax normalizer comes out of the PV matmul via a ones-column on V
  4. exp (no max subtraction needed: |scores| <= ~25), causal via
     skipping k>q column ranges + triangular 0/1 mask on diagonal blocks
  5. PV with V augmented by ones column -> unnormalized out + sumexp,
     normalize with reciprocal, reassemble OP (inverse .view), OP @ w_o
"""

import numpy as np

import concourse.bass as bass
import concourse.mybir as mybir
import concourse.tile as tile
from concourse import bacc
from concourse.bass_utils import run_bass_kernel_spmd
from concourse.masks import make_identity

B, S, D, H, DK = 8, 1024, 1024, 16, 64
P = 128
F32 = mybir.dt.float32
F32R = mybir.dt.float32r


HP_BUFS = 3


def _build_nc(mm_mode: str = "f32"):
    """Build the Bass program. mm_mode: 'f32' (exact) or 'f32r' (fast fp32)."""

    # In f32r mode, tiles feeding the big matmuls (projections, scores,
    # final) are declared float32r so their producers (ACT/DVE copies, exp)
    # round on write, which the BIR verifier requires. PV + transposes stay
    # full fp32.
    MDT = F32R if mm_mode == "f32r" else F32

    def mmc(ap):
        return ap

    nc = bacc.Bacc(
        "TRN2",
        target_bir_lowering=False,
        debug=False,
        enable_asserts=False,
        num_devices=B,
    )

    q_d = nc.dram_tensor("q", [S, D], F32, kind="ExternalInput")
    k_d = nc.dram_tensor("k", [S, D], F32, kind="ExternalInput")
    v_d = nc.dram_tensor("v", [S, D], F32, kind="ExternalInput")
    wq_d = nc.dram_tensor("w_q", [D, D], F32, kind="ExternalInput")
    wk_d = nc.dram_tensor("w_k", [D, D], F32, kind="ExternalInput")
    wv_d = nc.dram_tensor("w_v", [D, D], F32, kind="ExternalInput")
    wo_d = nc.dram_tensor("w_o", [D, D], F32, kind="ExternalInput")
    out_d = nc.dram_tensor("out", [S, D], F32, kind="ExternalOutput")

    qp_d = nc.dram_tensor("qp_scratch", [S, D], F32, kind="Internal")
    kp_d = nc.dram_tensor("kp_scratch", [S, D], F32, kind="Internal")
    vp_d = nc.dram_tensor("vp_scratch", [S, D], F32, kind="Internal")
    op_d = nc.dram_tensor("op_scratch", [S, D], F32, kind="Internal")

    with tile.TileContext(nc) as tc:
        with (
            tc.tile_pool(name="consts", bufs=1) as consts,
            tc.tile_pool(name="wpool", bufs=1) as wpool,
            tc.tile_pool(name="xtp", bufs=1) as xtp,
            tc.tile_pool(name="iop", bufs=3) as iop,
            tc.tile_pool(name="shp", bufs=2) as shp,
            tc.tile_pool(name="hp", bufs=HP_BUFS) as hp,
            tc.tile_pool(name="ptp", bufs=2) as ptp,
            tc.tile_pool(name="sp", bufs=4) as sp,
            tc.tile_pool(name="psA", bufs=3, space="PSUM") as psA,
            tc.tile_pool(name="psB", bufs=2, space="PSUM") as psB,
            tc.tile_pool(name="psC", bufs=3, space="PSUM") as psC,
        ):
            ident = consts.tile([P, P], F32, tag="ident")
            make_identity(nc, ident[:])
            # tri[k, q] = 1.0 if q >= k else 0.0  (keep causal-valid entries)
            tri = consts.tile([P, P], F32, tag="tri")
            nc.gpsimd.memset(tri[:], 1.0)
            nc.gpsimd.affine_select(
                out=tri[:],
                in_=tri[:],
                compare_op=mybir.AluOpType.is_ge,
                fill=0.0,
                base=0,
                pattern=[[1, P]],
                channel_multiplier=-1,
            )

            # ---------------- Phase 1: projections -> DRAM scratch ----------
            for x_d, w_d, xp_d in (
                (q_d, wq_d, qp_d),
                (k_d, wk_d, kp_d),
                (v_d, wv_d, vp_d),
            ):
                w_sb = wpool.tile([P, 8, 1024], MDT, tag="w")
                if MDT == F32:
                    nc.sync.dma_start(
                        w_sb[:], w_d.ap().rearrange("(kd p) c -> p kd c", p=P)
                    )
                else:
                    for kd in range(8):
                        wstg = iop.tile([P, 1024], F32, tag="wstg")
                        nc.sync.dma_start(wstg[:], w_d.ap()[kd * P : (kd + 1) * P, :])
                        nc.scalar.copy(out=w_sb[:, kd, :], in_=wstg[:])
                xt_sb = xtp.tile([P, 8, 1024], MDT, tag="xt")
                for st in range(8):
                    nat = iop.tile([P, 1024], F32, tag="nat")
                    nc.sync.dma_start(nat[:], x_d.ap()[st * P : (st + 1) * P, :])
                    for kd in range(8):
                        tp = psB.tile([P, P], F32, tag="tp")
                        nc.tensor.transpose(
                            tp[:], nat[:, kd * P : (kd + 1) * P], ident[:]
                        )
                        nc.vector.tensor_copy(
                            out=xt_sb[:, kd, st * P : (st + 1) * P], in_=tp[:]
                        )
                for st in range(8):
                    for ch in range(2):
                        ps = psA.tile([P, 512], F32, tag="mm")
                        for kd in range(8):
                            nc.tensor.matmul(
                                ps[:],
                                mmc(xt_sb[:, kd, st * P : (st + 1) * P]),
                                mmc(w_sb[:, kd, ch * 512 : (ch + 1) * 512]),
                                start=(kd == 0),
                                stop=(kd == 7),
                            )
                        stg = iop.tile([P, 512], F32, tag="stg")
                        nc.vector.tensor_copy(out=stg[:], in_=ps[:])
                        nc.scalar.dma_start(
                            xp_d.ap()[
                                st * P : (st + 1) * P, ch * 512 : (ch + 1) * 512
                            ],
                            stg[:],
                        )

            # ------------- Phase 2: attention, one head at a time -----------
            # X_h[s, dk] = XP[h*64 + s//16, (s%16)*64 + dk] -> contiguous view
            qp_r = qp_d.ap().rearrange("(h a) (b u) -> h (a b) u", h=H, b=16)
            kp_r = kp_d.ap().rearrange("(h a) (b u) -> h (a b) u", h=H, b=16)
            vp_r = vp_d.ap().rearrange("(h a) (b u) -> h (a b) u", h=H, b=16)

            # op merge target: [h, pa, pb, i, u] view of OP[r, c] with
            # r = h*64 + i*8 + pa, c = pb*64 + u  (inverse torch-.view)
            op_w = op_d.ap().rearrange(
                "(hh i pa) (pb u) -> hh pa pb i u", i=8, pa=8, pb=16
            )

            for hp2 in range(H // 2):
                h0 = 2 * hp2
                # pair tiles: partitions 0-63 = head h0's dk, 64-127 = h0+1's
                qT2 = hp.tile([P, 1024], MDT, tag="qhT")
                kT2 = hp.tile([P, 1024], MDT, tag="khT")
                qh2 = shp.tile([P, 8, P], F32, tag="qh")
                kh2 = shp.tile([P, 8, P], F32, tag="kh")
                for hh in range(2):
                    nc.sync.dma_start(
                        qh2[:, :, hh * DK : (hh + 1) * DK],
                        qp_r[h0 + hh].rearrange("(t p) u -> p t u", p=P),
                    )
                    nc.scalar.dma_start(
                        kh2[:, :, hh * DK : (hh + 1) * DK],
                        kp_r[h0 + hh].rearrange("(t p) u -> p t u", p=P),
                    )
                for t in range(8):
                    tpq = psB.tile([P, P], F32, tag="tp")
                    nc.tensor.transpose(tpq[:], qh2[:, t, :], ident[:])
                    nc.vector.tensor_copy(
                        out=qT2[:, t * P : (t + 1) * P], in_=tpq[:]
                    )
                    tpk = psB.tile([P, P], F32, tag="tp")
                    nc.tensor.transpose(tpk[:], kh2[:, t, :], ident[:])
                    nc.vector.tensor_copy(
                        out=kT2[:, t * P : (t + 1) * P], in_=tpk[:]
                    )

                for hh in range(2):
                    h = h0 + hh
                    r0, r1 = hh * DK, (hh + 1) * DK
                    vo = hp.tile([P, 8, DK + 1], F32, tag="vo")
                    if h < HP_BUFS:
                        # slots recycle per-tag; the ones column persists
                        nc.vector.memset(vo[:, :, DK : DK + 1], 1.0)
                    nc.gpsimd.dma_start(
                        vo[:, :, :DK], vp_r[h].rearrange("(t p) u -> p t u", p=P)
                    )

                    # packed causal P^T: row block j holds q in [j*128, 1024)
                    pt = ptp.tile([P, 4608], F32, tag="pt")
                    ptoff = [j * 1024 - 64 * j * (j - 1) for j in range(9)]
                    for j in range(8):
                        q0 = j * P
                        off = q0
                        while off < 1024:
                            n = min(512, 1024 - off)
                            ps = psA.tile([P, 512], F32, tag="mm")
                            nc.tensor.matmul(
                                ps[:, :n],
                                mmc(kT2[r0:r1, q0 : q0 + P]),
                                mmc(qT2[r0:r1, off : off + n]),
                                start=True,
                                stop=True,
                            )
                            nc.scalar.activation(
                                out=pt[:, ptoff[j] + off - q0 : ptoff[j] + off - q0 + n],
                                in_=ps[:, :n],
                                func=mybir.ActivationFunctionType.Exp,
                                scale=0.125,
                            )
                            off += n
                        # causal mask inside the diagonal 128x128 block
                        nc.vector.tensor_tensor(
                            pt[:, ptoff[j] : ptoff[j] + P],
                            pt[:, ptoff[j] : ptoff[j] + P],
                            tri[:],
                            mybir.AluOpType.mult,
                        )

                    hs = hp.tile([P, 8, DK], F32, tag="hs")
                    for i in range(8):
                        pv = psC.tile([P, DK + 1], F32, tag="pv")
                        for j in range(i + 1):
                            nc.tensor.matmul(
                                pv[:],
                                mmc(pt[:, ptoff[j] + (i - j) * P : ptoff[j] + (i - j + 1) * P]),
                                mmc(vo[:, j, :]),
                                start=(j == 0),
                                stop=(j == i),
                            )
                        rec = sp.tile([P, 1], F32, tag="rec")
                        nc.vector.reciprocal(rec[:], pv[:, DK : DK + 1])
                        nc.vector.tensor_scalar_mul(hs[:, i, :], pv[:, :DK], rec[:])
                    # one merge DMA per head: source (p,i,u) pairs with
                    # dest (pa,pb,i,u) in identical flat order
                    nc.gpsimd.dma_start(op_w[h], hs[:])

            # ---------------- Phase 3: output projection --------------------
            wo_sb = wpool.tile([P, 8, 1024], MDT, tag="w")
            if MDT == F32:
                nc.sync.dma_start(
                    wo_sb[:], wo_d.ap().rearrange("(kd p) c -> p kd c", p=P)
                )
            else:
                for kd in range(8):
                    wstg = iop.tile([P, 1024], F32, tag="wstg")
                    nc.sync.dma_start(wstg[:], wo_d.ap()[kd * P : (kd + 1) * P, :])
                    nc.scalar.copy(out=wo_sb[:, kd, :], in_=wstg[:])
            opT = xtp.tile([P, 8, 1024], MDT, tag="xt")
            for m in range(8):
                opn = iop.tile([P, 1024], F32, tag="nat")
                nc.sync.dma_start(opn[:], op_d.ap()[m * P : (m + 1) * P, :])
                for cc in range(8):
                    tp = psB.tile([P, P], F32, tag="tp")
                    nc.tensor.transpose(
                        tp[:], opn[:, cc * P : (cc + 1) * P], ident[:]
                    )
                    nc.vector.tensor_copy(
                        out=opT[:, cc, m * P : (m + 1) * P], in_=tp[:]
                    )
            for mt in range(8):
                for ch in range(2):
                    ps = psA.tile([P, 512], F32, tag="mm")
                    for cd in range(8):
                        nc.tensor.matmul(
                            ps[:],
                            mmc(opT[:, cd, mt * P : (mt + 1) * P]),
                            mmc(wo_sb[:, cd, ch * 512 : (ch + 1) * 512]),
                            start=(cd == 0),
                            stop=(cd == 7),
                        )
                    stg = iop.tile([P, 512], F32, tag="stg")
                    nc.vector.tensor_copy(out=stg[:], in_=ps[:])
                    nc.scalar.dma_start(
                        out_d.ap()[
                            mt * P : (mt + 1) * P, ch * 512 : (ch + 1) * 512
                        ],
                        stg[:],
                    )

    if not nc.is_finalized():
        nc.finalize()
    return nc


_nc_cache = {}


def _get_nc(mm_mode):
    if mm_mode not in _nc_cache:
        _nc_cache[mm_mode] = _build_nc(mm_mode)
    return _nc_cache[mm_mode]


MM_MODE = "f32"


def kernel(q, k, v, mask, w_q, w_k, w_v, w_o, _trace=False):
    q = np.ascontiguousarray(np.asarray(q, dtype=np.float32))
    k = np.ascontiguousarray(np.asarray(k, dtype=np.float32))
    v = np.ascontiguousarray(np.asarray(v, dtype=np.float32))
    w_q = np.ascontiguousarray(np.asarray(w_q, dtype=np.float32))
    w_k = np.ascontiguousarray(np.asarray(w_k, dtype=np.float32))
    w_v = np.ascontiguousarray(np.asarray(w_v, dtype=np.float32))
    w_o = np.ascontiguousarray(np.asarray(w_o, dtype=np.float32))

    nc = _get_nc(MM_MODE)
    in_maps = [
        {
            "q": q[i],
            "k": k[i],
            "v": v[i],
            "w_q": w_q,
            "w_k": w_k,
            "w_v": w_v,
            "w_o": w_o,
        }
        for i in range(B)
    ]
    res = run_bass_kernel_spmd(
        nc, in_maps, core_ids=list(range(B)), trace=_trace
    )
    out = np.stack([r["out"] for r in res.results], axis=0)
    if _trace:
        kernel.last_exec_time_ns = res.exec_time_ns
        kernel.last_trace = res.instructions_and_trace
    return out
